# revision 3
# baseline (speedup 1.0000x reference)
"""Multi-head causal attention (GPT-2 style) on 8 TRN2 NeuronCores.

Sharding: core i handles batch i//2 and head-group i%2 (8 of 16 heads,
i.e. a 512-wide slice of the QKV projections and of the Wp rows).  Each
core computes a partial output-projection for its batch; partials from
the two cores of a batch are summed on the host (cheap 4MB adds), along
with the exactly-factored bias terms:
  - bq is added to Q on-device (affects scores per key-column),
  - bk is dropped (adds a per-query constant to scores: softmax-invariant),
  - bv and bp commute through attention (rows of attn sum to 1):
    y += bv @ Wp + bp, applied on host.

On-chip layout (per core), T=1024, C=1024, DH=64:
  xT   [C, T]   x transposed (host-side transpose)         -> rhs / lhsT
  Q^T  [512, T] = (Wq_s*s)^T x^T  (s=1/8 folded into Wq)   -> scores rhs
  K^T  [512, T]                                            -> scores lhsT
  V    [T, 8, 65] natural layout + ones column             -> ctx lhsT
  S^T  [k-tile 128, q-chunk 512] scores transposed: softmax denominator
       comes out of the ctx matmul via the ones column of V; causal mask
       applied as an elementwise multiply on exp(S^T) diagonal blocks.
  ctx^T[512, T] normalized context                         -> yproj lhsT
All matmuls run in float32r (1 cycle/row on the PE at N=512; ~1e-4
relative accuracy), accumulation in fp32 PSUM.
"""
import numpy as np

import concourse.bacc as bacc
import concourse.mybir as mybir
import concourse.tile as tile
from concourse.bass_utils import run_bass_kernel_spmd

B, T, C, H, DH = 4, 1024, 1024, 16, 64
P = 128
CS = 512            # per-core head-slice width (8 heads * 64)
F32 = mybir.dt.float32
F32R = mybir.dt.float32r
AF = mybir.ActivationFunctionType
N_CORES = 8


def build_nc():
    nc = bacc.Bacc("TRN2", target_bir_lowering=False, debug=False,
                   num_devices=N_CORES)
    xT = nc.dram_tensor("xT", [C, T], F32R, kind="ExternalInput")
    wq = nc.dram_tensor("wq", [C, CS], F32R, kind="ExternalInput")
    wk = nc.dram_tensor("wk", [C, CS], F32R, kind="ExternalInput")
    wv = nc.dram_tensor("wv", [C, CS], F32R, kind="ExternalInput")
    wp = nc.dram_tensor("wp", [CS, C], F32R, kind="ExternalInput")
    bq = nc.dram_tensor("bq", [P, 4], F32, kind="ExternalInput")
    mask = nc.dram_tensor("mask", [4, P, 512], F32, kind="ExternalInput")
    ones = nc.dram_tensor("ones", [P, 64], F32R, kind="ExternalInput")
    y = nc.dram_tensor("y", [T, C], F32, kind="ExternalOutput")

    def r(ap):
        return ap

    with tile.TileContext(nc) as tc:
        with (
            tc.tile_pool(name="big", bufs=1) as big,
            tc.tile_pool(name="es_pool", bufs=4) as es_pool,
            tc.tile_pool(name="y_pool", bufs=3) as y_pool,
            tc.tile_pool(name="small", bufs=2) as small,
            tc.tile_pool(name="proj_ps", bufs=2, space="PSUM") as proj_ps,
            tc.tile_pool(name="sc_ps", bufs=2, space="PSUM") as sc_ps,
            tc.tile_pool(name="ctx_ps", bufs=2, space="PSUM") as ctx_ps,
        ):
            xT_sb = big.tile([P, 8, T], F32R)
            wq_sb = big.tile([P, 8, CS], F32R)
            wk_sb = big.tile([P, 8, CS], F32R)
            wv_sb = big.tile([P, 8, CS], F32R)
            wp_sb = big.tile([P, 4, C], F32R)
            bq_sb = big.tile([P, 4], F32)
            mask_sb = big.tile([P, 4, 512], F32)
            qT_sb = big.tile([P, 4, 2, 512], F32R)
            kT_sb = big.tile([P, 4, 2, 512], F32R)
            v_sb = big.tile([P, 8, 8, 65], F32R)
            ctxT_sb = big.tile([P, 4, T], F32R)

            nc.sync.dma_start(out=xT_sb, in_=xT.ap().rearrange("(c p) t -> p c t", p=P))
            nc.sync.dma_start(out=wq_sb, in_=wq.ap().rearrange("(c p) n -> p c n", p=P))
            nc.sync.dma_start(out=wk_sb, in_=wk.ap().rearrange("(c p) n -> p c n", p=P))
            nc.sync.dma_start(out=wv_sb, in_=wv.ap().rearrange("(c p) n -> p c n", p=P))
            nc.sync.dma_start(out=wp_sb, in_=wp.ap().rearrange("(k p) n -> p k n", p=P))
            nc.sync.dma_start(out=bq_sb, in_=bq.ap())
            nc.sync.dma_start(out=mask_sb, in_=mask.ap().rearrange("r p j -> p r j"))

            # ---- Phase 1: projections ----
            # Q^T, K^T: [512, T], computed as (W)^T @ x^T chunkwise.
            for wsb, outsb, is_q in ((wq_sb, qT_sb, True), (wk_sb, kT_sb, False)):
                for mc in range(4):
                    for tc2 in range(2):
                        ps = proj_ps.tile([P, 512], F32, tag="proj")
                        for c in range(8):
                            nc.tensor.matmul(
                                ps,
                                r(wsb[:, c, mc * P:(mc + 1) * P]),
                                r(xT_sb[:, c, tc2 * 512:(tc2 + 1) * 512]),
                                start=(c == 0), stop=(c == 7),
                            )
                        dst = outsb[:, mc, tc2, :]
                        if is_q:
                            nc.scalar.activation(dst, ps, AF.Identity,
                                                 bias=bq_sb[:, mc:mc + 1])
                        else:
                            nc.vector.tensor_copy(dst, ps)

            # V natural [T, 512] + ones column per head (DMA'd: memset
            # cannot target float32r).
            nc.sync.dma_start(out=v_sb[:, :, :, 64],
                              in_=ones.ap().rearrange("p (a b) -> p a b", a=8))
            for tt in range(8):
                ps = proj_ps.tile([P, 512], F32, tag="proj")
                for c in range(8):
                    nc.tensor.matmul(
                        ps,
                        r(xT_sb[:, c, tt * P:(tt + 1) * P]),
                        r(wv_sb[:, c, :]),
                        start=(c == 0), stop=(c == 7),
                    )
                nc.vector.tensor_copy(
                    v_sb[:, tt, :, 0:64],
                    ps.rearrange("p (h d) -> p h d", h=8),
                )

            # ---- Phase 2: attention per head ----
            for h in range(8):
                hp = (h % 2) * 64
                mc = h // 2
                for qc in range(2):
                    nkt = 4 * (qc + 1)
                    cps = ctx_ps.tile([65, 512], F32, tag="ctx")
                    for kt in range(nkt):
                        sps = sc_ps.tile([P, 512], F32, tag="sc")
                        nc.tensor.matmul(
                            sps,
                            r(kT_sb[hp:hp + 64, mc, kt // 4,
                                    (kt % 4) * P:(kt % 4 + 1) * P]),
                            r(qT_sb[hp:hp + 64, mc, qc, :]),
                            start=True, stop=True,
                        )
                        es = es_pool.tile([P, 512], F32R, tag="es")
                        nc.scalar.activation(es, sps, AF.Exp)
                        rdiag = kt - qc * 4
                        if rdiag >= 0:
                            nc.vector.tensor_mul(es, es, mask_sb[:, rdiag, :])
                        nc.tensor.matmul(
                            cps,
                            r(v_sb[:, kt, h, :]),
                            r(es),
                            start=(kt == 0), stop=(kt == nkt - 1),
                        )
                    recip = small.tile([1, 512], F32, tag="recip")
                    nc.vector.reciprocal(recip, cps[64:65, :])
                    recb = small.tile([64, 512], F32, tag="recb")
                    nc.gpsimd.partition_broadcast(recb, recip)
                    nc.vector.tensor_mul(
                        ctxT_sb[hp:hp + 64, mc, qc * 512:(qc + 1) * 512],
                        cps[0:64, :], recb)

            # ---- Phase 3: output projection (partial y) ----
            for tt in range(8):
                for nk in range(2):
                    ps = proj_ps.tile([P, 512], F32, tag="proj")
                    for kc in range(4):
                        nc.tensor.matmul(
                            ps,
                            r(ctxT_sb[:, kc, tt * P:(tt + 1) * P]),
                            r(wp_sb[:, kc, nk * 512:(nk + 1) * 512]),
                            start=(kc == 0), stop=(kc == 3),
                        )
                    ysb = y_pool.tile([P, 512], F32, tag="y")
                    nc.vector.tensor_copy(ysb, ps)
                    nc.sync.dma_start(
                        out=y.ap()[tt * P:(tt + 1) * P, nk * 512:(nk + 1) * 512],
                        in_=ysb)
    nc.compile()
    return nc


_NC = None


def _get_nc():
    global _NC
    if _NC is None:
        _NC = build_nc()
    return _NC


def make_in_maps(x, Wq, bq, Wk, Wv, Wp):
    """Per-core input dicts (fp32 numpy)."""
    jj = np.arange(512)[None, :]
    pp = np.arange(P)[:, None]
    masks = np.stack([(jj >= pp + rr * P) for rr in range(4)]).astype(np.float32)
    in_maps = []
    for core in range(N_CORES):
        b = core // 2
        g = core % 2
        cs = slice(g * CS, (g + 1) * CS)
        in_maps.append(dict(
            xT=np.ascontiguousarray(x[b].T),
            wq=np.ascontiguousarray(Wq[:, cs]) * np.float32(0.125),
            wk=np.ascontiguousarray(Wk[:, cs]),
            wv=np.ascontiguousarray(Wv[:, cs]),
            wp=np.ascontiguousarray(Wp[cs, :]),
            bq=np.ascontiguousarray((bq[cs] * np.float32(0.125))
                                    .reshape(4, P).T),
            mask=masks,
            ones=np.ones((P, 64), np.float32),
        ))
    return in_maps


def combine(parts, Wq, bv, Wp, bp):
    """parts: list of 8 per-core partial y arrays -> full [B, T, C] output."""
    out = np.stack([parts[2 * b] + parts[2 * b + 1] for b in range(B)])
    out += (bv @ Wp + bp)[None, None, :]
    return out.astype(np.float32)


def kernel(**inputs):
    x = np.asarray(inputs["x"], np.float32)
    Wq = np.asarray(inputs["Wq"], np.float32)
    bq = np.asarray(inputs["bq"], np.float32)
    Wk = np.asarray(inputs["Wk"], np.float32)
    Wv = np.asarray(inputs["Wv"], np.float32)
    Wp = np.asarray(inputs["Wp"], np.float32)
    bv = np.asarray(inputs["bv"], np.float32)
    bp = np.asarray(inputs["bp"], np.float32)
    # bk intentionally unused: it shifts every score of a query row by the
    # same amount, which softmax cancels exactly.

    nc = _get_nc()
    in_maps = make_in_maps(x, Wq, bq, Wk, Wv, Wp)
    res = run_bass_kernel_spmd(nc, in_maps, core_ids=list(range(N_CORES)))
    parts = [res.results[c]["y"] for c in range(N_CORES)]
    return combine(parts, Wq, bv, Wp, bp)
